# revision 2
# baseline (speedup 1.0000x reference)
"""TRN2 Bass kernel: multiresolution hash-grid encode + SIREN MLPs.

Strategy (data-parallel over 8 NeuronCores, per sharding hint):
  - input_points [524288, 3] sharded along N into 8 x 65536; hash table
    (8 levels x 2^19 x 4 f32, 64MB) and tiny SIREN weights replicated.
  - Phase 1 (per core, For_i over 32 batches of 2048 points): DVE computes
    the spatial-hash corner indices exactly in int32 (the XOR hash is
    separable: idx = x ^ (y*P2 mod 2^19) ^ (z*P3 mod 2^19); the per-axis
    products are built from exact fp32 pieces), per-corner features are
    fetched with indirect DMA gathers (128 rows of 16B per instruction),
    trilinear-interpolated on DVE, and the [128,32] enc tiles are
    PE-transposed to a [32, n] layout stashed in DRAM.
  - Phase 2: SIREN on PE (fp32 matmuls, activations stay [feat, batch] so
    no per-layer transposes) + ACT Sin (bias pre-scaled by omega on host).
Outputs: scalar [N,1] f32 and density [N] f32 (matches reference tuple).
"""
import sys

sys.path.insert(0, '/opt/trn_rl_repo')
from contextlib import ExitStack

import numpy as np

import concourse.bass as bass
import concourse.tile as tile
from concourse import bacc, mybir
from concourse.bass import IndirectOffsetOnAxis
from concourse.bass_utils import run_bass_kernel_spmd

FP = mybir.dt.float32
I32 = mybir.dt.int32
ALU = mybir.AluOpType
ACTF = mybir.ActivationFunctionType

L = 8
T = 2 ** 19
P2 = 2654435761
P3 = 805459861
OMEGA = 30.0
N_TOTAL = 524288
N_CORES = 8
N_CORE = N_TOTAL // N_CORES
BATCH = 2048
_growth = np.exp((np.log(2.0 ** 12) - np.log(16.0)) / (L - 1))
RES = np.floor(16.0 * _growth ** np.arange(L)).astype(np.float32)

p19_2 = P2 % T
A2, B2 = float(p19_2 // 1024), float(p19_2 % 1024)
p19_3 = P3 % T
A3, B3 = float(p19_3 // 1024), float(p19_3 % 1024)

S1_SHAPES = [(32, 64), (64, 64), (64, 64), (64, 64), (64, 64), (64, 16)]
S2_SHAPES = [(47, 64), (64, 64), (64, 64), (64, 1)]


def _build(n_core, batch):
    nb = n_core // batch
    Q = batch // 128
    nc = bacc.Bacc()

    pts_d = nc.dram_tensor("pts", [n_core, 3], FP, kind="ExternalInput")
    tab_d = nc.dram_tensor("tab", [L * T, 4], FP, kind="ExternalInput")
    ident_d = nc.dram_tensor("ident", [128, 128], FP, kind="ExternalInput")
    res_d = nc.dram_tensor("resv", [128, L], FP, kind="ExternalInput")
    lb_d = nc.dram_tensor("lbv", [128, L], FP, kind="ExternalInput")
    w1_d = [nc.dram_tensor(f"w1_{i}", list(s), FP, kind="ExternalInput") for i, s in enumerate(S1_SHAPES)]
    b1_d = [nc.dram_tensor(f"b1_{i}", [s[1], 1], FP, kind="ExternalInput") for i, s in enumerate(S1_SHAPES)]
    w2_d = [nc.dram_tensor(f"w2_{i}", list(s), FP, kind="ExternalInput") for i, s in enumerate(S2_SHAPES)]
    b2_d = [nc.dram_tensor(f"b2_{i}", [s[1], 1], FP, kind="ExternalInput") for i, s in enumerate(S2_SHAPES)]
    scal_d = nc.dram_tensor("scal", [1, n_core], FP, kind="ExternalOutput")
    dens_d = nc.dram_tensor("dens", [1, n_core], FP, kind="ExternalOutput")

    with tile.TileContext(nc) as tc:
        with ExitStack() as ctx:
            cpool = ctx.enter_context(tc.tile_pool(name="consts", bufs=1))
            dpool = ctx.enter_context(tc.tile_pool(name="drams", bufs=1, space="DRAM"))
            encT_dram = dpool.tile([32, n_core], FP)

            res_t = cpool.tile([128, L], FP)
            nc.sync.dma_start(res_t[:], res_d[:])
            lb_t = cpool.tile([128, L], FP)
            nc.sync.dma_start(lb_t[:], lb_d[:])
            ident = cpool.tile([128, 128], FP)
            nc.sync.dma_start(ident[:], ident_d[:])
            w1_t, b1_t, w2_t, b2_t = [], [], [], []
            for i, s in enumerate(S1_SHAPES):
                w = cpool.tile([s[0], s[1]], FP, tag=f"w1_{i}")
                nc.sync.dma_start(w[:], w1_d[i][:])
                w1_t.append(w)
                b = cpool.tile([s[1], 1], FP, tag=f"b1_{i}")
                nc.sync.dma_start(b[:], b1_d[i][:])
                b1_t.append(b)
            for i, s in enumerate(S2_SHAPES):
                w = cpool.tile([s[0], s[1]], FP, tag=f"w2_{i}")
                nc.sync.dma_start(w[:], w2_d[i][:])
                w2_t.append(w)
                b = cpool.tile([s[1], 1], FP, tag=f"b2_{i}")
                nc.sync.dma_start(b[:], b2_d[i][:])
                b2_t.append(b)

            # ---------------- phase 1: hash encode ----------------
            with ExitStack() as p1:
                pool = p1.enter_context(tc.tile_pool(name="p1", bufs=1))
                pspool = p1.enter_context(tc.tile_pool(name="p1ps", bufs=2, space="PSUM"))

                def bcL(ap_128L, shape):
                    a = ap_128L
                    for _ in range(len(shape) - 2):
                        a = a.unsqueeze(1)
                    return a.broadcast_to(shape)

                with tc.For_i(0, nb, hint_engines=(mybir.EngineType.Pool,)) as bi:
                    pts_t = pool.tile([128, Q, 3], FP)
                    src = pts_d[bass.ts(bi, batch), :].rearrange("(q p) c -> p q c", p=128)
                    nc.sync.dma_start(pts_t[:], src)

                    t01 = pool.tile([128, Q, 3], FP)
                    nc.vector.tensor_scalar(t01[:], pts_t[:], 1.0, 0.5, ALU.add, ALU.mult)
                    p_all = pool.tile([128, Q, 3, L], FP)
                    nc.vector.tensor_tensor(
                        p_all[:], t01[:].unsqueeze(3).broadcast_to([128, Q, 3, L]),
                        bcL(res_t[:], [128, Q, 3, L]), ALU.mult)
                    # floor(p), robust to either cast rounding mode:
                    # r = roundcast(p); floor = r - (r > p)
                    ri = pool.tile([128, Q, 3, L], I32)
                    nc.vector.tensor_copy(ri[:], p_all[:])
                    rf = pool.tile([128, Q, 3, L], FP)
                    nc.vector.tensor_copy(rf[:], ri[:])
                    gt = pool.tile([128, Q, 3, L], FP)
                    nc.vector.tensor_tensor(gt[:], rf[:], p_all[:], ALU.is_gt)
                    c0f = pool.tile([128, Q, 3, L], FP)
                    nc.vector.tensor_tensor(c0f[:], rf[:], gt[:], ALU.subtract)
                    w_all = pool.tile([128, Q, 3, L], FP)
                    nc.vector.tensor_tensor(w_all[:], p_all[:], c0f[:], ALU.subtract)

                    xf = c0f[:, :, 0, :]
                    yf = c0f[:, :, 1, :]
                    zf = c0f[:, :, 2, :]
                    shQ = [128, Q, L]

                    def axhash(src_ap, A, B, P19I, pfx):
                        # (v * p19) mod 2^19 via exact fp32/int pieces (v <= 4096
                        # integral): p19 = A*1024 + B.
                        t1f = pool.tile(shQ, FP, tag=pfx + "t1f")
                        nc.vector.tensor_scalar(t1f[:], src_ap, A, None, ALU.mult)
                        t1i = pool.tile(shQ, I32, tag=pfx + "t1i")
                        nc.vector.tensor_copy(t1i[:], t1f[:])
                        t1m = pool.tile(shQ, I32, tag=pfx + "t1m")
                        nc.vector.tensor_scalar(t1m[:], t1i[:], 511, None, ALU.bitwise_and)
                        t1a = pool.tile(shQ, I32, tag=pfx + "t1a")
                        nc.vector.tensor_scalar(t1a[:], t1m[:], 1024, None, ALU.mult)
                        t2f = pool.tile(shQ, FP, tag=pfx + "t2f")
                        nc.vector.tensor_scalar(t2f[:], src_ap, B, None, ALU.mult)
                        t2i = pool.tile(shQ, I32, tag=pfx + "t2i")
                        nc.vector.tensor_copy(t2i[:], t2f[:])
                        g0s = pool.tile(shQ, I32, tag=pfx + "g0s")
                        nc.vector.tensor_tensor(g0s[:], t1a[:], t2i[:], ALU.add)
                        g0 = pool.tile(shQ, I32, tag=pfx + "g0")
                        nc.vector.tensor_scalar(g0[:], g0s[:], T - 1, None, ALU.bitwise_and)
                        g1a = pool.tile(shQ, I32, tag=pfx + "g1a")
                        nc.vector.tensor_scalar(g1a[:], g0[:], P19I, None, ALU.add)
                        g1 = pool.tile(shQ, I32, tag=pfx + "g1")
                        nc.vector.tensor_scalar(g1[:], g1a[:], T - 1, None, ALU.bitwise_and)
                        return g0, g1

                    gy0i, gy1i = axhash(yf, A2, B2, p19_2, "y_")
                    hz0i, hz1i = axhash(zf, A3, B3, p19_3, "z_")
                    # x corner values with the per-level table base folded in
                    # (base = l*2^19, disjoint from the 19 hash bits)
                    xl0 = pool.tile(shQ, FP)
                    nc.vector.tensor_tensor(xl0[:], xf, bcL(lb_t[:], shQ), ALU.add)
                    xl1 = pool.tile(shQ, FP)
                    nc.vector.tensor_scalar(xl1[:], xl0[:], 1.0, None, ALU.add)

                    def toi(src_t, tag):
                        t = pool.tile(shQ, I32, tag=tag)
                        nc.vector.tensor_copy(t[:], src_t[:])
                        return t

                    xi = [toi(xl0, "xi0"), toi(xl1, "xi1")]
                    gyi = [gy0i, gy1i]
                    hzi = [hz0i, hz1i]
                    m = {}
                    for j in range(2):
                        for k in range(2):
                            mt = pool.tile(shQ, I32, tag=f"m{j}{k}")
                            nc.vector.tensor_tensor(mt[:], gyi[j][:], hzi[k][:], ALU.bitwise_xor)
                            m[(j, k)] = mt
                    idx_all = pool.tile([128, Q, L, 8], I32)
                    for i in range(2):
                        for j in range(2):
                            for k in range(2):
                                c = i * 4 + j * 2 + k
                                nc.vector.tensor_tensor(
                                    idx_all[:, :, :, c], xi[i][:], m[(j, k)][:], ALU.bitwise_xor)

                    wx = w_all[:, :, 0, :]
                    wy = w_all[:, :, 1, :]
                    wz = w_all[:, :, 2, :]
                    wneg = pool.tile([128, Q, 3, L], FP)
                    nc.vector.tensor_scalar(wneg[:], w_all[:], -1.0, 1.0, ALU.mult, ALU.add)
                    ux = wneg[:, :, 0, :]
                    uy = wneg[:, :, 1, :]
                    uz = wneg[:, :, 2, :]
                    wyz = {}
                    for j in range(2):
                        for k in range(2):
                            t = pool.tile(shQ, FP, tag=f"wyz{j}{k}")
                            nc.vector.tensor_tensor(t[:], (wy if j else uy), (wz if k else uz), ALU.mult)
                            wyz[(j, k)] = t
                    wc_all = pool.tile([128, Q, L, 8], FP)
                    for i in range(2):
                        for j in range(2):
                            for k in range(2):
                                c = i * 4 + j * 2 + k
                                nc.vector.tensor_tensor(
                                    wc_all[:, :, :, c], (wx if i else ux), wyz[(j, k)][:], ALU.mult)

                    # gathers: one SWDGE instruction per (q, l, c): 128 offsets,
                    # one 16B row per partition
                    feats = pool.tile([128, Q, L, 8, 4], FP)
                    for q in range(Q):
                        for l in range(L):
                            for c in range(8):
                                nc.gpsimd.indirect_dma_start(
                                    feats[:, q, l, c], None, tab_d[:],
                                    IndirectOffsetOnAxis(ap=idx_all[:, q, l, c:c + 1], axis=0))

                    wf = pool.tile([128, Q, L, 8, 4], FP)
                    nc.vector.tensor_tensor(
                        wf[:], feats[:],
                        wc_all[:].unsqueeze(4).broadcast_to([128, Q, L, 8, 4]), ALU.mult)
                    enc = pool.tile([128, Q, L, 4], FP)
                    nc.vector.tensor_reduce(
                        enc[:], wf[:].rearrange("p q l c f -> p q l f c"),
                        axis=mybir.AxisListType.X, op=ALU.add)

                    encT_sb = pool.tile([32, Q * 128], FP)
                    for q in range(Q):
                        ps_t = pspool.tile([32, 128], FP, tag="tps")
                        nc.tensor.transpose(ps_t[:], enc[:, q, :, :], ident[:])
                        nc.scalar.activation(encT_sb[:, q * 128:(q + 1) * 128], ps_t[:], ACTF.Copy)
                    nc.sync.dma_start(encT_dram[:, bass.ts(bi, batch)], encT_sb[:])

            tc.strict_bb_all_engine_barrier()

            # ---------------- phase 2: SIREN ----------------
            with ExitStack() as p2:
                pool = p2.enter_context(tc.tile_pool(name="p2", bufs=2))
                hpool = p2.enter_context(tc.tile_pool(name="p2h", bufs=3))
                pspool = p2.enter_context(tc.tile_pool(name="p2ps", bufs=4, space="PSUM"))
                for b in range(n_core // batch):
                    cols = slice(b * batch, (b + 1) * batch)
                    enc_sb = pool.tile([32, batch], FP, tag="enc_sb")
                    nc.sync.dma_start(enc_sb[:], encT_dram[:, cols])
                    x2in = pool.tile([47, batch], FP, tag="x2in")
                    nc.sync.dma_start(x2in[15:47, :], encT_dram[:, cols])
                    dens_sb = pool.tile([1, batch], FP, tag="dens_sb")
                    scal_sb = pool.tile([1, batch], FP, tag="scal_sb")
                    for n in range(batch // 512):
                        ncols = slice(n * 512, (n + 1) * 512)
                        h = enc_sb[:, ncols]
                        for li in range(6):
                            ps = pspool.tile([64, 512], FP, tag="ps")
                            nc.tensor.matmul(ps[:S1_SHAPES[li][1], :], w1_t[li][:], h, start=True, stop=True)
                            if li < 5:
                                hn = hpool.tile([64, 512], FP, tag=f"h{li % 2}")
                                nc.scalar.activation(hn[:], ps[:64, :], ACTF.Sin,
                                                     bias=b1_t[li][:], scale=OMEGA)
                                h = hn[:]
                            else:
                                nc.scalar.activation(dens_sb[:, ncols], ps[0:1, :], ACTF.Relu,
                                                     bias=b1_t[5][0:1, :], scale=1.0)
                                s1o = hpool.tile([16, 512], FP, tag="s1o")
                                nc.vector.tensor_scalar(s1o[:], ps[:16, :], b1_t[5][:], None, ALU.add)
                                nc.sync.dma_start(x2in[0:15, ncols], s1o[1:16, :])
                        h = x2in[:, ncols]
                        for li in range(4):
                            ps = pspool.tile([64, 512], FP, tag="ps")
                            nc.tensor.matmul(ps[:S2_SHAPES[li][1], :], w2_t[li][:], h, start=True, stop=True)
                            if li < 3:
                                hn = hpool.tile([64, 512], FP, tag=f"g{li % 2}")
                                nc.scalar.activation(hn[:], ps[:64, :], ACTF.Sin,
                                                     bias=b2_t[li][:], scale=OMEGA)
                                h = hn[:]
                            else:
                                nc.vector.tensor_scalar(scal_sb[:, ncols], ps[0:1, :], b2_t[3][:], None, ALU.add)
                    nc.sync.dma_start(dens_d[:, cols], dens_sb[:])
                    nc.sync.dma_start(scal_d[:, cols], scal_sb[:])

    nc.compile()
    return nc


_NC_CACHE = {}


def _get_nc():
    key = (N_CORE, BATCH)
    if key not in _NC_CACHE:
        _NC_CACHE[key] = _build(N_CORE, BATCH)
    return _NC_CACHE[key]


def kernel(input_points, table, s1_w, s1_b, s2_w, s2_b):
    input_points = np.asarray(input_points, np.float32)
    table = np.asarray(table, np.float32)
    assert input_points.shape == (N_TOTAL, 3)
    assert table.shape == (L, T, 4)

    shared = {
        "tab": np.ascontiguousarray(table.reshape(L * T, 4)),
        "ident": np.eye(128, dtype=np.float32),
        "resv": np.tile(RES.reshape(1, L), (128, 1)),
        "lbv": np.tile((np.arange(L, dtype=np.float32) * np.float32(T)).reshape(1, L), (128, 1)),
    }
    for i, (w, b) in enumerate(zip(s1_w, s1_b)):
        shared[f"w1_{i}"] = np.ascontiguousarray(np.asarray(w, np.float32))
        bb = np.asarray(b, np.float32).reshape(-1, 1)
        shared[f"b1_{i}"] = (bb * np.float32(OMEGA)) if i < 5 else bb
    for i, (w, b) in enumerate(zip(s2_w, s2_b)):
        shared[f"w2_{i}"] = np.ascontiguousarray(np.asarray(w, np.float32))
        bb = np.asarray(b, np.float32).reshape(-1, 1)
        shared[f"b2_{i}"] = (bb * np.float32(OMEGA)) if i < 3 else bb

    in_maps = []
    for c in range(N_CORES):
        m = dict(shared)
        m["pts"] = np.ascontiguousarray(input_points[c * N_CORE:(c + 1) * N_CORE])
        in_maps.append(m)

    nc = _get_nc()
    res = run_bass_kernel_spmd(nc, in_maps, core_ids=list(range(N_CORES)))

    scal = np.concatenate([np.asarray(r["scal"]).reshape(-1) for r in res.results])
    dens = np.concatenate([np.asarray(r["dens"]).reshape(-1) for r in res.results])
    return scal.reshape(N_TOTAL, 1).astype(np.float32), dens.astype(np.float32)
